# revision 14
# baseline (speedup 1.0000x reference)
"""FFF (fast feedforward / soft MoE tree) layer for Trainium2, 8 NeuronCores.

Strategy: data-parallel over the 4096-token batch (512 tokens/core), all
weights replicated. Per core, activations live feature-major in SBUF
([feature partitions, token free-dim]) so every matmul uses native weight
slices as lhsT and 512-token tiles as rhs:

  node phase:  hn^T = relu(W1n^T x^T + b1)          8 x 6 matmuls, N=512
               z    = W2bd2^T hn^T                  8 matmuls; the block-diag
                                                    W2 columns are DUPLICATED
                                                    (126-wide stationary) so
                                                    zp[0:126] holds z twice
               sp   = ln(1 + exp(+-z -+ b2))        softplus via exp+ln ACTs
                                                    (one per-partition
                                                    scale/bias exp, then ln
                                                    with bias=1; both tables
                                                    preloaded in node slack)
               w^T  = exp(Mpath^T sp)               fp32r path matmul + exp
  leaf phase:  per leaf l: hl = relu(W1_l^T x^T + b1_l)   6 matmuls -> PSUM
               hls = hl * w_l (per-token scale via broadcast DMA of w rows)
               out^T += W2_l^T @ hls                 6 accumulating matmuls
               (+ leaf_b2 folded in as a rank-64 matmul over w^T)

Head DMAs ride the Scalar engine's hardware DGE queue (Scalar exits the
NEFF prologue ~0.8us before Sync) ordered xt-c0, w1n-j0/j1, xt-rest so the
first node matmul starts as early as possible; w1n j2-7 rides Sync in
parallel. PE warmup burns the remaining DMA window with cheap bf16 dummy
matmuls (the HAM clock gate needs ~3.4us of sustained PE activity to
release 1.2 -> 2.4 GHz).

out^T [768, 512] accumulates in 6 PSUM banks across all 64 leaves (4-leaf
software-pipeline skew keeps the PE saturated; the final leaves drain
bank-major so PSUM->SBUF copies overlap the last matmuls), then three
batched DMAs write DRAM; the host transposes and concatenates the 8 core
shards. Matmul inputs are bf16 (fp32 accumulation in PSUM); the path
matmul runs fp32r (1 cycle/col at 512 cols) and all bias handling is fp32.
"""

import functools
import os
import sys
from contextlib import ExitStack

import numpy as np
import ml_dtypes

for _p in ("/opt/trn_rl_repo", "/root/.axon_site/_ro/trn_rl_repo"):
    if os.path.isdir(_p) and _p not in sys.path:
        sys.path.insert(0, _p)

import concourse.bass as bass
import concourse.tile as tile
from concourse import bacc, mybir
from concourse.bass_utils import run_bass_kernel_spmd

BF16 = ml_dtypes.bfloat16

DEPTH = 6
IN_DIM = 768
NODE_HIDDEN = 16
LEAF_HIDDEN = 128
OUT_DIM = 768
BATCH = 4096
N_NODES = 63
N_LEAVES = 64
N_CORES = 8
BC = BATCH // N_CORES          # 512 tokens per core
KC = IN_DIM // 128             # 6 contraction chunks
HN = N_NODES * NODE_HIDDEN     # 1008 node-hidden total
NJ = (HN + 127) // 128         # 8 node-hidden partition tiles (last = 112)
OC = OUT_DIM // 128            # 6 output-feature chunks
GL = 8                         # leaves per weight-DMA group (fewer DMA issues)
WG = 4                         # leaves per w-broadcast group
ZW = 2 * N_NODES               # 126: z duplicated across two partition bands

# Exposed for test harnesses.
LAST_RESULT = None


def _path_matrix() -> np.ndarray:
    """Mpath [128, 64] with -1 entries: logw = Mpath^T @ softplus-stack.

    sp row n (0..62) holds softplus(-z_n - b2_n) = -ln c_n; row 63+n holds
    softplus(z_n + b2_n) = -ln(1-c_n). Row n is selected (-1) for leaves in
    the LEFT subtree of node n, row 63+n for its RIGHT subtree, so
    Mpath^T @ sp = sum ln(gate) = ln w. Rows 126/127 are zero.
    """
    m = np.zeros((128, N_LEAVES), np.float32)
    for leaf in range(N_LEAVES):
        for lvl in range(DEPTH):
            node = (1 << lvl) - 1 + (leaf >> (DEPTH - lvl))
            right = (leaf >> (DEPTH - 1 - lvl)) & 1
            m[node + (N_NODES if right else 0), leaf] = -1.0
    return m


@functools.lru_cache(maxsize=1)
def _build_nc() -> bass.Bass:
    nc = bacc.Bacc()
    f32 = mybir.dt.float32
    f32r = mybir.dt.float32r
    bf16 = mybir.dt.bfloat16

    xt_d = nc.dram_tensor("xt", [128, KC, BC], bf16, kind="ExternalInput")
    w1n_d = nc.dram_tensor("w1n", [128, NJ, KC * 128], bf16, kind="ExternalInput")
    w2bd_d = nc.dram_tensor("w2bd", [128, NJ, ZW], bf16, kind="ExternalInput")
    b1n_d = nc.dram_tensor("b1n", [128, NJ], f32, kind="ExternalInput")
    # spsb: col 0 = softplus bias (-b2 rows 0:63, +b2 rows 63:126),
    #       col 1 = softplus scale (-1 rows 0:63, +1 rows 63:126)
    spsb_d = nc.dram_tensor("spsb", [128, 2], f32, kind="ExternalInput")
    mpath_d = nc.dram_tensor("mpath", [128, N_LEAVES], f32r, kind="ExternalInput")
    lw1_d = nc.dram_tensor(
        "lw1", [N_LEAVES // GL, 128, GL * KC * 128], bf16, kind="ExternalInput"
    )
    b1l_d = nc.dram_tensor("b1l", [128, N_LEAVES], f32, kind="ExternalInput")
    lw2_d = nc.dram_tensor(
        "lw2", [N_LEAVES // GL, 128, GL * OUT_DIM], bf16, kind="ExternalInput"
    )
    b2l_d = nc.dram_tensor("b2l", [N_LEAVES, OUT_DIM], bf16, kind="ExternalInput")
    out_d = nc.dram_tensor("outT", [OUT_DIM, BC], f32, kind="ExternalOutput")
    # Staging buffer so the per-token leaf weights can be broadcast-read
    # (partition-step-0 APs need a DRAM source).
    wt_dram = nc.dram_tensor("wt_scratch", [N_LEAVES, BC], bf16)

    act = mybir.ActivationFunctionType
    alu = mybir.AluOpType

    with tile.TileContext(nc) as tc, ExitStack() as ctx:
        consts = ctx.enter_context(tc.tile_pool(name="consts", bufs=1))
        wpool = ctx.enter_context(tc.tile_pool(name="wpool", bufs=3))
        apool = ctx.enter_context(tc.tile_pool(name="apool", bufs=2))
        ppool = ctx.enter_context(tc.tile_pool(name="ppool", bufs=2, space="PSUM"))
        opool = ctx.enter_context(tc.tile_pool(name="opool", bufs=1, space="PSUM"))

        xt = consts.tile([128, KC, BC], bf16)
        w1n = consts.tile([128, NJ, KC, 128], bf16)
        # Head DMAs on the Scalar hardware-DGE queue: Scalar leaves the
        # NEFF prologue ~0.8us before Sync, and the first node matmul needs
        # exactly xt-c0 + w1n-j0. w1n j2-7 rides Sync concurrently.
        nc.scalar.dma_start(out=xt[:, 0:1, :], in_=xt_d[:, 0:1, :])
        nc.scalar.dma_start(
            out=w1n[:, 0, :, :],
            in_=w1n_d[:, 0, :].rearrange("p (c h) -> p c h", c=KC),
        )
        nc.scalar.dma_start(
            out=w1n[:, 1, :, :],
            in_=w1n_d[:, 1, :].rearrange("p (c h) -> p c h", c=KC),
        )
        nc.scalar.dma_start(out=xt[:, 1:, :], in_=xt_d[:, 1:, :])
        nc.sync.dma_start(
            out=w1n[:, 2:NJ, :, :],
            in_=w1n_d[:, 2:NJ, :].rearrange("p j (c h) -> p j c h", c=KC),
        )
        # Small consts trickle over the (slow) GpSimd software queue,
        # ordered by first use time.
        b1n = consts.tile([128, NJ], f32)
        nc.gpsimd.dma_start(out=b1n, in_=b1n_d[:])
        w2bd = consts.tile([128, NJ, ZW], bf16)
        nc.gpsimd.dma_start(out=w2bd, in_=w2bd_d[:])
        spsb = consts.tile([128, 2], f32)
        nc.gpsimd.dma_start(out=spsb, in_=spsb_d[:])
        b1l = consts.tile([128, N_LEAVES], f32)
        nc.gpsimd.dma_start(out=b1l, in_=b1l_d[:])
        mpath = consts.tile([128, N_LEAVES], f32r)
        nc.gpsimd.dma_start(out=mpath, in_=mpath_d[:])
        b2l = consts.tile([N_LEAVES, OUT_DIM], bf16)
        nc.gpsimd.dma_start(out=b2l, in_=b2l_d[:])

        hn = consts.tile([128, NJ, BC], bf16)
        ue = consts.tile([128, BC], f32)
        # f32r so the path matmul streams at 1 cycle/col (fp32 is 4x slower)
        sp = consts.tile([128, BC], f32r)
        wt = consts.tile([N_LEAVES, BC], bf16)
        # rows 126/127 of sp stay 0 so Mpath's zero rows multiply finite data
        # (memset through an f32 view; the DVE can't write f32r directly)
        nc.vector.memset(sp[:, :].bitcast(f32), 0.0)
        wz = consts.tile([128, BC], bf16)
        nc.vector.memset(wz, 0.0)
        tli = consts.tile([128, 1], f32)
        nc.vector.memset(tli, 0.0)
        tlw = consts.tile([128, 1], f32, name="tlw")

        # PE warmup: the HAM clock gate keeps an idle PE at 1.2 GHz and only
        # releases to 2.4 GHz after ~3.4us of sustained activity. The PE sits
        # idle waiting for the head DMAs anyway, so burn that window with
        # cheap bf16 dummy matmuls (213ns each).
        warm = ppool.tile([128, BC], f32, tag="work", name="warm")
        for _ in range(10):
            nc.tensor.matmul(warm[:1, :], wz[:, :1], wz, start=True, stop=True)

        # ---- node phase: duplicated gate pre-activations z ----
        for j in range(NJ):
            pj = min(128, HN - j * 128)
            ph = ppool.tile([128, BC], f32, tag="work")
            for c in range(KC):
                nc.tensor.matmul(
                    ph[:pj, :],
                    w1n[:, j, c, :pj],
                    xt[:, c, :],
                    start=(c == 0),
                    stop=(c == KC - 1),
                )
            nc.scalar.activation(
                hn[:pj, j, :], ph[:pj, :], act.Relu, bias=b1n[:pj, j : j + 1]
            )
            if j == 0:
                # preload the Exp/Ln ACT tables during node-phase slack so
                # the gating chain's table traffic stays off its critical
                # path as much as the table-set model allows
                nc.scalar.activation(tlw, tli, act.Exp)
                nc.scalar.activation(tlw, tli, act.Ln, bias=1.0)

        zp = ppool.tile([128, BC], f32, tag="work")
        for j in range(NJ):
            pj = min(128, HN - j * 128)
            nc.tensor.matmul(
                zp[:ZW, :],
                w2bd[:pj, j, :],
                hn[:pj, j, :],
                start=(j == 0),
                stop=(j == NJ - 1),
            )
        # Softplus from exp+ln (no softplus table on TRN2):
        # ue[n]    = exp(-z_n - b2_n);  ue[63+n] = exp(+z_n + b2_n)
        # sp[n]    = ln(1 + ue[n])    = -ln c_n        (left subtrees)
        # sp[63+n] = ln(1 + ue[63+n]) = -ln(1 - c_n)   (right subtrees)
        # Gate pre-activations are O(10), so ue stays far from fp32 inf.
        nc.scalar.activation(
            ue[:ZW, :], zp[:ZW, :], act.Exp,
            bias=spsb[:ZW, 0:1], scale=spsb[:ZW, 1:2],
        )
        nc.scalar.activation(sp[:ZW, :], ue[:ZW, :], act.Ln, bias=1.0)

        # ---- leaf-phase pipeline helpers ----
        pouts = [
            opool.tile([128, BC], f32, tag=f"out{o}", name=f"pout{o}")
            for o in range(OC)
        ]
        wreps = {}

        def emit_wrep_dma(grp):
            """Broadcast leaf-weight rows (4 leaves) across all partitions."""
            wrep = wpool.tile([128, WG, BC], bf16, tag="wrep", bufs=3, name="wrep")
            src = bass.AP(
                tensor=wt_dram,
                offset=grp * WG * BC,
                ap=[[0, 128], [BC, WG], [1, BC]],
            )
            nc.sync.dma_start(out=wrep, in_=src)
            wreps[grp] = wrep

        lwg = {}

        def emit_lw1_dma(g):
            w1t = wpool.tile([128, GL, KC, 128], bf16, tag="lw1", bufs=2, name="w1t")
            nc.sync.dma_start(
                out=w1t,
                in_=lw1_d[g].rearrange("p (i c h) -> p i c h", i=GL, c=KC),
            )
            return w1t

        def emit_lw2_dma(g):
            w2t = wpool.tile([128, GL, OUT_DIM], bf16, tag="lw2", bufs=2, name="w2t")
            nc.sync.dma_start(
                out=w2t, in_=lw2_d[g].rearrange("p (i o) -> p i o", i=GL)
            )
            return w2t

        def front_a(leaf, defer_lw2=False, dve_relu=False):
            """Weight DMAs (grouped) + hl matmuls + relu for one leaf."""
            if leaf % WG == 2 and leaf >= WG and leaf + 2 < N_LEAVES:
                # prefetch the NEXT group's broadcast two leaves early
                # (groups 0/1 are emitted explicitly after wt_dram is
                # written; emission order carries the RAW dep on wt_dram)
                emit_wrep_dma(leaf // WG + 1)
            g = leaf // GL
            if leaf % GL == 0:
                lwg[g] = [emit_lw1_dma(g), None if defer_lw2 else emit_lw2_dma(g)]
            w1t = lwg[g][0]
            i = leaf % GL

            ph = ppool.tile([128, BC], f32, tag="work", name="ph")
            for c in range(KC):
                nc.tensor.matmul(
                    ph,
                    w1t[:, i, c, :],
                    xt[:, c, :],
                    start=(c == 0),
                    stop=(c == KC - 1),
                )
            hl = apool.tile([128, BC], bf16, tag="hl", bufs=6, name="hl")
            if dve_relu:
                # DVE relu keeps the Scalar queue free for the gating chain
                nc.vector.tensor_scalar(
                    hl, ph, b1l[:, leaf : leaf + 1], 0.0, alu.add, alu.max
                )
            else:
                nc.scalar.activation(
                    hl, ph, act.Relu, bias=b1l[:, leaf : leaf + 1]
                )
            return (hl, leaf)

        def front_b(st):
            """Per-token leaf-weight scale (needs wrep for the leaf's group)."""
            hl, leaf = st
            hls = apool.tile([128, BC], bf16, tag="hls", bufs=7, name="hls")
            nc.vector.tensor_mul(hls, hl, wreps[leaf // WG][:, leaf % WG, :])
            return (hls, leaf)

        def leaf_out(pend, last=False):
            p_hls, p_leaf = pend
            p_w2t = lwg[p_leaf // GL][1]
            for o in range(OC):
                nc.tensor.matmul(
                    pouts[o],
                    p_w2t[:, p_leaf % GL, o * 128 : (o + 1) * 128],
                    p_hls,
                    start=False,
                    stop=last,
                )

        # Prefill leaves: their hl matmuls keep PE busy while the gating
        # chain (softplus -> path matmul -> exp -> DRAM round trip for the
        # broadcast) produces the leaf weights. Group 0's second-matmul
        # weights are deferred so the wt round trip doesn't queue behind
        # their transfer.
        prefill = [front_a(0, defer_lw2=True, dve_relu=True),
                   front_a(1, dve_relu=True)]

        lwp = ppool.tile([128, BC], f32, tag="work", name="lwp")
        nc.tensor.matmul(
            lwp[:N_LEAVES, :], mpath, sp, start=True, stop=True
        )
        nc.scalar.activation(wt, lwp[:N_LEAVES, :], act.Exp)
        nc.sync.dma_start(out=wt_dram[:], in_=wt)
        emit_wrep_dma(0)
        emit_wrep_dma(1)

        # more prefilled leaves cover the exp -> wt -> wrep round trip
        prefill += [front_a(2, dve_relu=True), front_a(3, dve_relu=True),
                    front_a(4)]
        lwg[0][1] = emit_lw2_dma(0)
        pending = [front_b(st) for st in prefill]

        # leaf_b2 contribution: out^T += b2l^T @ w^T (starts the accumulation)
        for o in range(OC):
            nc.tensor.matmul(
                pouts[o], b2l[:, o * 128 : (o + 1) * 128], wt, start=True, stop=False
            )

        # steady state: 4-leaf software-pipeline skew
        for leaf in range(5, N_LEAVES):
            pending.append(front_b(front_a(leaf)))
            leaf_out(pending.pop(0))

        # Final leaves drain BANK-major: each output bank finishes all its
        # remaining accumulations consecutively, then its PSUM->SBUF copy
        # starts while later banks are still accumulating. Output DMAs ship
        # in three pieces (3/2/1 banks) so the LAST bank -- the critical
        # path after the final matmul -- rides a small 256KB transfer.
        osb = apool.tile([128, OC, BC], f32, tag="osb", bufs=1, name="osb")
        dma_after = {2: (0, 3), 4: (3, 5), 5: (5, 6)}
        for o in range(OC):
            for idx, (p_hls, p_leaf) in enumerate(pending):
                nc.tensor.matmul(
                    pouts[o],
                    lwg[p_leaf // GL][1][:, p_leaf % GL, o * 128 : (o + 1) * 128],
                    p_hls,
                    start=False,
                    stop=(idx == len(pending) - 1),
                )
            nc.vector.tensor_copy(osb[:, o, :], pouts[o])
            if o in dma_after:
                lo, hi = dma_after[o]
                nc.sync.dma_start(
                    out=out_d[lo * 128 : hi * 128, :].rearrange(
                        "(o p) t -> p o t", p=128
                    ),
                    in_=osb[:, lo:hi, :],
                )

    nc.compile()
    return nc


def _to_bf16(a: np.ndarray) -> np.ndarray:
    return np.ascontiguousarray(a, dtype=np.float32).astype(BF16)


def prep_inputs(x, node_w1, node_b1, node_w2, node_b2,
                leaf_w1, leaf_b1, leaf_w2, leaf_b2):
    """Host-side layout prep. Returns (shared weight map, per-core x maps)."""
    x = np.asarray(x, np.float32)
    node_w1 = np.asarray(node_w1, np.float32)
    node_b1 = np.asarray(node_b1, np.float32)
    node_w2 = np.asarray(node_w2, np.float32)
    node_b2 = np.asarray(node_b2, np.float32)
    leaf_w1 = np.asarray(leaf_w1, np.float32)
    leaf_b1 = np.asarray(leaf_b1, np.float32)
    leaf_w2 = np.asarray(leaf_w2, np.float32)
    leaf_b2 = np.asarray(leaf_b2, np.float32)

    # node W1 -> [128, NJ, KC*128]: (p, j, c*128+h') = W1n[c*128+p, j*128+h']
    # (W1n [768, 1008] zero-padded to 1024 columns)
    w1n_flat = node_w1.transpose(1, 0, 2).reshape(IN_DIM, HN)
    w1n_pad = np.zeros((IN_DIM, NJ * 128), np.float32)
    w1n_pad[:, :HN] = w1n_flat
    w1n = w1n_pad.reshape(KC, 128, NJ, 128).transpose(1, 2, 0, 3)
    w1n = w1n.reshape(128, NJ, KC * 128)
    # block-diagonal node W2 [HN, 126] with columns DUPLICATED (n and 63+n),
    # padded to 1024 rows -> [128, NJ, 126]
    w2bd = np.zeros((NJ * 128, ZW), np.float32)
    for n in range(N_NODES):
        w2bd[n * NODE_HIDDEN : (n + 1) * NODE_HIDDEN, n] = node_w2[n, :, 0]
        w2bd[n * NODE_HIDDEN : (n + 1) * NODE_HIDDEN, N_NODES + n] = node_w2[n, :, 0]
    w2bd = w2bd.reshape(NJ, 128, ZW).transpose(1, 0, 2)
    # node b1 -> [128, NJ]
    b1n = np.zeros((NJ * 128,), np.float32)
    b1n[:HN] = node_b1.reshape(-1)
    b1n = b1n.reshape(NJ, 128).T
    # softplus bias/scale stack: rows 0:63 -> (-b2, -1), rows 63:126 -> (+b2, +1)
    b2 = node_b2[:, 0]
    spsb = np.zeros((128, 2), np.float32)
    spsb[:N_NODES, 0] = -b2
    spsb[:N_NODES, 1] = -1.0
    spsb[N_NODES : 2 * N_NODES, 0] = b2
    spsb[N_NODES : 2 * N_NODES, 1] = 1.0

    # leaf W1 grouped GL leaves per DMA: [NG, 128, GL*KC*128] with
    # (g, p, (i, c, h)) = leaf_w1[g*GL+i, c*128+p, h]
    ng = N_LEAVES // GL
    lw1 = leaf_w1.reshape(ng, GL, KC, 128, LEAF_HIDDEN).transpose(0, 3, 1, 2, 4)
    lw1 = lw1.reshape(ng, 128, GL * KC * 128)
    # leaf W2 grouped: [NG, 128, GL*OUT] with (g, p, (i, o)) = leaf_w2[g*GL+i, p, o]
    lw2 = leaf_w2.reshape(ng, GL, LEAF_HIDDEN, OUT_DIM).transpose(0, 2, 1, 3)
    lw2 = lw2.reshape(ng, 128, GL * OUT_DIM)
    b1l = leaf_b1.T  # [128, 64]

    shared = {
        "w1n": _to_bf16(w1n),
        "w2bd": _to_bf16(w2bd),
        "b1n": np.ascontiguousarray(b1n, np.float32),
        "spsb": spsb,
        "mpath": _path_matrix(),
        "lw1": _to_bf16(lw1),
        "b1l": np.ascontiguousarray(b1l, np.float32),
        "lw2": _to_bf16(lw2),
        "b2l": _to_bf16(leaf_b2),
    }
    xts = []
    for c in range(N_CORES):
        xc = x[c * BC : (c + 1) * BC].T  # [768, 512]
        xt = xc.reshape(KC, 128, BC).transpose(1, 0, 2)
        xts.append(_to_bf16(xt))
    return shared, xts


def kernel(**inputs) -> np.ndarray:
    global LAST_RESULT
    shared, xts = prep_inputs(**inputs)
    nc = _build_nc()
    in_maps = [{**shared, "xt": xts[c]} for c in range(N_CORES)]
    trace = os.environ.get("FFF_TRACE", "0") == "1"
    res = run_bass_kernel_spmd(nc, in_maps, list(range(N_CORES)), trace=trace)
    LAST_RESULT = res
    out = np.empty((BATCH, OUT_DIM), np.float32)
    for c in range(N_CORES):
        out[c * BC : (c + 1) * BC, :] = res.results[c]["outT"].T
    return out


# revision 18
# speedup vs baseline: 1.1069x; 1.1069x over previous
"""FFF (fast feedforward / soft MoE tree) layer for Trainium2, 8 NeuronCores.

Strategy: data-parallel over the 4096-token batch (512 tokens/core), all
weights replicated. Per core, activations live feature-major in SBUF
([feature partitions, token free-dim]) so every matmul uses native weight
slices as lhsT and 512-token tiles as rhs:

  node phase:  hn^T = relu(W1n^T x^T + b1)          8 x 6 matmuls, N=512
               z    = W2bd2^T hn^T                  8 matmuls; the block-diag
                                                    W2 columns are DUPLICATED
                                                    (126-wide stationary) so
                                                    zp[0:126] holds z twice
               sp   = ln(1 + exp(+-z -+ b2))        softplus via exp+ln ACTs
                                                    (one per-partition
                                                    scale/bias exp, then ln
                                                    with bias=1; both tables
                                                    preloaded in node slack)
               w^T  = exp(Mpath^T sp)               fp32r path matmul + exp
  leaf phase:  per leaf l: hl = relu(W1_l^T x^T + b1_l)   6 matmuls -> PSUM
               hls = hl * w_l (per-token scale via broadcast DMA of w rows)
               out^T += W2_l^T @ hls                 6 accumulating matmuls
               (+ leaf_b2 folded in as a rank-64 matmul over w^T)

Head DMAs ride the Scalar engine's hardware DGE queue (Scalar exits the
NEFF prologue ~0.8us before Sync) ordered xt-c0, w1n-j0/j1, xt-rest so the
first node matmul starts as early as possible; w1n j2-7 rides Sync in
parallel. PE warmup burns the remaining DMA window with cheap bf16 dummy
matmuls (the HAM clock gate needs ~3.4us of sustained PE activity to
release 1.2 -> 2.4 GHz).

out^T [768, 512] accumulates in 6 PSUM banks across all 64 leaves (4-leaf
software-pipeline skew keeps the PE saturated; the final leaves drain
bank-major so PSUM->SBUF copies overlap the last matmuls), then three
batched DMAs write DRAM; the host transposes and concatenates the 8 core
shards. Matmul inputs are bf16 (fp32 accumulation in PSUM); the path
matmul runs fp32r (1 cycle/col at 512 cols) and all bias handling is fp32.
"""

import functools
import os
import sys
from contextlib import ExitStack

import numpy as np
import ml_dtypes

for _p in ("/opt/trn_rl_repo", "/root/.axon_site/_ro/trn_rl_repo"):
    if os.path.isdir(_p) and _p not in sys.path:
        sys.path.insert(0, _p)

import concourse.bass as bass
import concourse.tile as tile
from concourse import bacc, mybir
from concourse.bass_utils import run_bass_kernel_spmd

# The act-table pass models a single current table set and reloads (1.3us
# on Scalar) at every function-set switch, choosing the FIRST set listing
# each function. The gating chain alternates exp -> ln -> exp, which the
# default choice (exp_and_others / natural_log) turns into three in-chain
# reloads. Hide exp/ln from every set except the combined
# natural_log_exp_and_others so the chooser lands on it for all three ACTs:
# one load total, zero switches. Set IDs are positional, so only the
# function SETS are edited in the bass-side view -- never reordered --
# keeping InstLoadActFuncSet ids aligned with the compiler's act_info.json.
_ORIG_GET_ACT_TABLES = bacc.get_activation_tables


@functools.lru_cache(maxsize=4)
def _patched_act_tables(arch):
    tables = _ORIG_GET_ACT_TABLES(arch)
    combined = tables.get("natural_log_exp_and_others")
    exp = mybir.ActivationFunctionType.Exp
    ln = mybir.ActivationFunctionType.Ln
    if combined and exp in combined and ln in combined:
        for name, fns in tables.items():
            if name != "natural_log_exp_and_others":
                fns.discard(exp)
                fns.discard(ln)
    return tables


bacc.get_activation_tables = _patched_act_tables

BF16 = ml_dtypes.bfloat16

DEPTH = 6
IN_DIM = 768
NODE_HIDDEN = 16
LEAF_HIDDEN = 128
OUT_DIM = 768
BATCH = 4096
N_NODES = 63
N_LEAVES = 64
N_CORES = 8
BC = BATCH // N_CORES          # 512 tokens per core
KC = IN_DIM // 128             # 6 contraction chunks
HN = N_NODES * NODE_HIDDEN     # 1008 node-hidden total
NJ = (HN + 127) // 128         # 8 node-hidden partition tiles (last = 112)
OC = OUT_DIM // 128            # 6 output-feature chunks
GL = 8                         # leaves per weight-DMA group (fewer DMA issues)
WG = 4                         # leaves per w-broadcast group
ZW = 2 * N_NODES               # 126: z duplicated across two partition bands

# Exposed for test harnesses.
LAST_RESULT = None


def _path_matrix() -> np.ndarray:
    """Mpath [128, 64] with -1 entries: logw = Mpath^T @ softplus-stack.

    sp row n (0..62) holds softplus(-z_n - b2_n) = -ln c_n; row 63+n holds
    softplus(z_n + b2_n) = -ln(1-c_n). Row n is selected (-1) for leaves in
    the LEFT subtree of node n, row 63+n for its RIGHT subtree, so
    Mpath^T @ sp = sum ln(gate) = ln w. Rows 126/127 are zero.
    """
    m = np.zeros((128, N_LEAVES), np.float32)
    for leaf in range(N_LEAVES):
        for lvl in range(DEPTH):
            node = (1 << lvl) - 1 + (leaf >> (DEPTH - lvl))
            right = (leaf >> (DEPTH - 1 - lvl)) & 1
            m[node + (N_NODES if right else 0), leaf] = -1.0
    return m


@functools.lru_cache(maxsize=1)
def _build_nc() -> bass.Bass:
    nc = bacc.Bacc()
    f32 = mybir.dt.float32
    f32r = mybir.dt.float32r
    bf16 = mybir.dt.bfloat16

    xt_d = nc.dram_tensor("xt", [128, KC, BC], bf16, kind="ExternalInput")
    w1n_d = nc.dram_tensor("w1n", [128, NJ, KC * 128], bf16, kind="ExternalInput")
    w2bd_d = nc.dram_tensor("w2bd", [128, NJ, ZW], bf16, kind="ExternalInput")
    b1n_d = nc.dram_tensor("b1n", [128, NJ], f32, kind="ExternalInput")
    # spsb: col 0 = softplus bias (-b2 rows 0:63, +b2 rows 63:126),
    #       col 1 = softplus scale (-1 rows 0:63, +1 rows 63:126)
    spsb_d = nc.dram_tensor("spsb", [128, 2], f32, kind="ExternalInput")
    mpath_d = nc.dram_tensor("mpath", [128, N_LEAVES], f32r, kind="ExternalInput")
    lw1_d = nc.dram_tensor(
        "lw1", [N_LEAVES // GL, 128, GL * KC * 128], bf16, kind="ExternalInput"
    )
    b1l_d = nc.dram_tensor("b1l", [128, N_LEAVES], f32, kind="ExternalInput")
    lw2_d = nc.dram_tensor(
        "lw2", [N_LEAVES // GL, 128, GL * OUT_DIM], bf16, kind="ExternalInput"
    )
    b2l_d = nc.dram_tensor("b2l", [N_LEAVES, OUT_DIM], bf16, kind="ExternalInput")
    out_d = nc.dram_tensor("outT", [OUT_DIM, BC], f32, kind="ExternalOutput")
    # Staging buffer so the per-token leaf weights can be broadcast-read
    # (partition-step-0 APs need a DRAM source).
    wt_dram = nc.dram_tensor("wt_scratch", [N_LEAVES, BC], bf16)

    act = mybir.ActivationFunctionType
    alu = mybir.AluOpType

    with tile.TileContext(nc) as tc, ExitStack() as ctx:
        consts = ctx.enter_context(tc.tile_pool(name="consts", bufs=1))
        wpool = ctx.enter_context(tc.tile_pool(name="wpool", bufs=3))
        apool = ctx.enter_context(tc.tile_pool(name="apool", bufs=2))
        ppool = ctx.enter_context(tc.tile_pool(name="ppool", bufs=2, space="PSUM"))
        opool = ctx.enter_context(tc.tile_pool(name="opool", bufs=1, space="PSUM"))

        xt = consts.tile([128, KC, BC], bf16)
        w1n = consts.tile([128, NJ, KC, 128], bf16)
        # Head DMAs all ride the Sync hardware-DGE queue (the Scalar-fed
        # queue starts later and trickles): the first node matmul needs
        # exactly xt-c0 + w1n-j0, so those two lead.
        nc.sync.dma_start(out=xt[:, 0:1, :], in_=xt_d[:, 0:1, :])
        nc.sync.dma_start(
            out=w1n[:, 0, :, :],
            in_=w1n_d[:, 0, :].rearrange("p (c h) -> p c h", c=KC),
        )
        nc.sync.dma_start(
            out=w1n[:, 1, :, :],
            in_=w1n_d[:, 1, :].rearrange("p (c h) -> p c h", c=KC),
        )
        nc.sync.dma_start(out=xt[:, 1:, :], in_=xt_d[:, 1:, :])
        nc.sync.dma_start(
            out=w1n[:, 2:NJ, :, :],
            in_=w1n_d[:, 2:NJ, :].rearrange("p j (c h) -> p j c h", c=KC),
        )
        # Small consts trickle over the (slow) GpSimd software queue,
        # ordered by first use time.
        b1n = consts.tile([128, NJ], f32)
        nc.gpsimd.dma_start(out=b1n, in_=b1n_d[:])
        w2bd = consts.tile([128, NJ, ZW], bf16)
        nc.gpsimd.dma_start(out=w2bd, in_=w2bd_d[:])
        spsb = consts.tile([128, 2], f32)
        nc.gpsimd.dma_start(out=spsb, in_=spsb_d[:])
        b1l = consts.tile([128, N_LEAVES], f32)
        nc.gpsimd.dma_start(out=b1l, in_=b1l_d[:])
        mpath = consts.tile([128, N_LEAVES], f32r)
        nc.gpsimd.dma_start(out=mpath, in_=mpath_d[:])
        b2l = consts.tile([N_LEAVES, OUT_DIM], bf16)
        nc.gpsimd.dma_start(out=b2l, in_=b2l_d[:])

        hn = consts.tile([128, NJ, BC], bf16)
        ue = consts.tile([128, BC], f32)
        # f32r so the path matmul streams at 1 cycle/col (fp32 is 4x slower)
        sp = consts.tile([128, BC], f32r)
        wt = consts.tile([N_LEAVES, BC], bf16)
        # rows 126/127 of sp stay 0 so Mpath's zero rows multiply finite data
        # (memset through an f32 view; the DVE can't write f32r directly)
        nc.vector.memset(sp[:, :].bitcast(f32), 0.0)
        wz = consts.tile([128, BC], bf16)
        nc.vector.memset(wz, 0.0)
        tli = consts.tile([128, 1], f32)
        nc.vector.memset(tli, 0.0)
        tlw = consts.tile([128, 1], f32, name="tlw")

        # PE warmup: the HAM clock gate keeps an idle PE at 1.2 GHz and only
        # releases to 2.4 GHz after ~3.4us of sustained activity. The PE sits
        # idle waiting for the head DMAs anyway, so burn that window with
        # cheap bf16 dummy matmuls (213ns each).
        warm = ppool.tile([128, BC], f32, tag="work", name="warm")
        for _ in range(22):
            nc.tensor.matmul(warm[:1, :], wz[:, :1], wz, start=True, stop=True)

        # ---- node phase: duplicated gate pre-activations z ----
        for j in range(NJ):
            pj = min(128, HN - j * 128)
            ph = ppool.tile([128, BC], f32, tag="work")
            for c in range(KC):
                nc.tensor.matmul(
                    ph[:pj, :],
                    w1n[:, j, c, :pj],
                    xt[:, c, :],
                    start=(c == 0),
                    stop=(c == KC - 1),
                )
            nc.scalar.activation(
                hn[:pj, j, :], ph[:pj, :], act.Relu, bias=b1n[:pj, j : j + 1]
            )
            if j == 0:
                # preload the combined exp+ln ACT table set (see the
                # act-table patch above) during node-phase slack; the
                # gating chain then runs with zero table loads
                nc.scalar.activation(tlw, tli, act.Exp)

        zp = ppool.tile([128, BC], f32, tag="work")
        for j in range(NJ):
            pj = min(128, HN - j * 128)
            nc.tensor.matmul(
                zp[:ZW, :],
                w2bd[:pj, j, :],
                hn[:pj, j, :],
                start=(j == 0),
                stop=(j == NJ - 1),
            )
        # Softplus from exp+ln (no softplus table on TRN2):
        # ue[n]    = exp(-z_n - b2_n);  ue[63+n] = exp(+z_n + b2_n)
        # sp[n]    = ln(1 + ue[n])    = -ln c_n        (left subtrees)
        # sp[63+n] = ln(1 + ue[63+n]) = -ln(1 - c_n)   (right subtrees)
        # Gate pre-activations are O(10), so ue stays far from fp32 inf.
        nc.scalar.activation(
            ue[:ZW, :], zp[:ZW, :], act.Exp,
            bias=spsb[:ZW, 0:1], scale=spsb[:ZW, 1:2],
        )
        nc.scalar.activation(sp[:ZW, :], ue[:ZW, :], act.Ln, bias=1.0)

        # ---- leaf-phase pipeline helpers ----
        pouts = [
            opool.tile([128, BC], f32, tag=f"out{o}", name=f"pout{o}")
            for o in range(OC)
        ]
        wreps = {}

        def emit_wrep_dma(grp):
            """Broadcast leaf-weight rows (4 leaves) across all partitions."""
            wrep = wpool.tile([128, WG, BC], bf16, tag="wrep", bufs=3, name="wrep")
            src = bass.AP(
                tensor=wt_dram,
                offset=grp * WG * BC,
                ap=[[0, 128], [BC, WG], [1, BC]],
            )
            nc.sync.dma_start(out=wrep, in_=src)
            wreps[grp] = wrep

        lwg = {}

        def emit_lw1_dma(g):
            w1t = wpool.tile([128, GL, KC, 128], bf16, tag="lw1", bufs=2, name="w1t")
            nc.sync.dma_start(
                out=w1t,
                in_=lw1_d[g].rearrange("p (i c h) -> p i c h", i=GL, c=KC),
            )
            return w1t

        def emit_lw2_dma(g):
            w2t = wpool.tile([128, GL, OUT_DIM], bf16, tag="lw2", bufs=2, name="w2t")
            nc.sync.dma_start(
                out=w2t, in_=lw2_d[g].rearrange("p (i o) -> p i o", i=GL)
            )
            return w2t

        def front_a(leaf, defer_lw2=False, dve_relu=False):
            """Weight DMAs (grouped) + hl matmuls + relu for one leaf."""
            if leaf % WG == 2 and leaf >= WG and leaf + 2 < N_LEAVES:
                # prefetch the NEXT group's broadcast two leaves early
                # (groups 0/1 are emitted explicitly after wt_dram is
                # written; emission order carries the RAW dep on wt_dram)
                emit_wrep_dma(leaf // WG + 1)
            g = leaf // GL
            if leaf % GL == 0:
                lwg[g] = [emit_lw1_dma(g), None if defer_lw2 else emit_lw2_dma(g)]
            w1t = lwg[g][0]
            i = leaf % GL

            ph = ppool.tile([128, BC], f32, tag="work", name="ph")
            for c in range(KC):
                nc.tensor.matmul(
                    ph,
                    w1t[:, i, c, :],
                    xt[:, c, :],
                    start=(c == 0),
                    stop=(c == KC - 1),
                )
            hl = apool.tile([128, BC], bf16, tag="hl", bufs=6, name="hl")
            if dve_relu:
                # DVE relu keeps the Scalar queue free for the gating chain
                nc.vector.tensor_scalar(
                    hl, ph, b1l[:, leaf : leaf + 1], 0.0, alu.add, alu.max
                )
            else:
                nc.scalar.activation(
                    hl, ph, act.Relu, bias=b1l[:, leaf : leaf + 1]
                )
            return (hl, leaf)

        def front_b(st):
            """Per-token leaf-weight scale (needs wrep for the leaf's group)."""
            hl, leaf = st
            hls = apool.tile([128, BC], bf16, tag="hls", bufs=7, name="hls")
            nc.vector.tensor_mul(hls, hl, wreps[leaf // WG][:, leaf % WG, :])
            return (hls, leaf)

        def leaf_out(pend, last=False):
            p_hls, p_leaf = pend
            p_w2t = lwg[p_leaf // GL][1]
            for o in range(OC):
                nc.tensor.matmul(
                    pouts[o],
                    p_w2t[:, p_leaf % GL, o * 128 : (o + 1) * 128],
                    p_hls,
                    start=False,
                    stop=last,
                )

        # Prefill leaves: their hl matmuls keep PE busy while the gating
        # chain (softplus -> path matmul -> exp -> DRAM round trip for the
        # broadcast) produces the leaf weights. Group 0's second-matmul
        # weights are deferred so the wt round trip doesn't queue behind
        # their transfer.
        prefill = [front_a(0, defer_lw2=True, dve_relu=True),
                   front_a(1, dve_relu=True)]

        lwp = ppool.tile([128, BC], f32, tag="work", name="lwp")
        nc.tensor.matmul(
            lwp[:N_LEAVES, :], mpath, sp, start=True, stop=True
        )
        nc.scalar.activation(wt, lwp[:N_LEAVES, :], act.Exp)
        nc.sync.dma_start(out=wt_dram[:], in_=wt)
        emit_wrep_dma(0)
        emit_wrep_dma(1)

        # more prefilled leaves cover the exp -> wt -> wrep round trip
        prefill += [front_a(2, dve_relu=True), front_a(3, dve_relu=True),
                    front_a(4)]
        lwg[0][1] = emit_lw2_dma(0)
        pending = [front_b(st) for st in prefill]

        # leaf_b2 contribution: out^T += b2l^T @ w^T (starts the accumulation)
        for o in range(OC):
            nc.tensor.matmul(
                pouts[o], b2l[:, o * 128 : (o + 1) * 128], wt, start=True, stop=False
            )

        # steady state: 4-leaf software-pipeline skew
        for leaf in range(5, N_LEAVES):
            pending.append(front_b(front_a(leaf)))
            leaf_out(pending.pop(0))

        # Final leaves drain BANK-major: each output bank finishes all its
        # remaining accumulations consecutively, then its PSUM->SBUF copy
        # starts while later banks are still accumulating. Output DMAs ship
        # in three pieces (3/2/1 banks) so the LAST bank -- the critical
        # path after the final matmul -- rides a small 256KB transfer.
        osb = apool.tile([128, OC, BC], f32, tag="osb", bufs=1, name="osb")
        dma_after = {2: (0, 3), 4: (3, 5), 5: (5, 6)}
        for o in range(OC):
            for idx, (p_hls, p_leaf) in enumerate(pending):
                nc.tensor.matmul(
                    pouts[o],
                    lwg[p_leaf // GL][1][:, p_leaf % GL, o * 128 : (o + 1) * 128],
                    p_hls,
                    start=False,
                    stop=(idx == len(pending) - 1),
                )
            nc.vector.tensor_copy(osb[:, o, :], pouts[o])
            if o in dma_after:
                lo, hi = dma_after[o]
                nc.sync.dma_start(
                    out=out_d[lo * 128 : hi * 128, :].rearrange(
                        "(o p) t -> p o t", p=128
                    ),
                    in_=osb[:, lo:hi, :],
                )

    nc.compile()
    return nc


def _to_bf16(a: np.ndarray) -> np.ndarray:
    return np.ascontiguousarray(a, dtype=np.float32).astype(BF16)


def prep_inputs(x, node_w1, node_b1, node_w2, node_b2,
                leaf_w1, leaf_b1, leaf_w2, leaf_b2):
    """Host-side layout prep. Returns (shared weight map, per-core x maps)."""
    x = np.asarray(x, np.float32)
    node_w1 = np.asarray(node_w1, np.float32)
    node_b1 = np.asarray(node_b1, np.float32)
    node_w2 = np.asarray(node_w2, np.float32)
    node_b2 = np.asarray(node_b2, np.float32)
    leaf_w1 = np.asarray(leaf_w1, np.float32)
    leaf_b1 = np.asarray(leaf_b1, np.float32)
    leaf_w2 = np.asarray(leaf_w2, np.float32)
    leaf_b2 = np.asarray(leaf_b2, np.float32)

    # node W1 -> [128, NJ, KC*128]: (p, j, c*128+h') = W1n[c*128+p, j*128+h']
    # (W1n [768, 1008] zero-padded to 1024 columns)
    w1n_flat = node_w1.transpose(1, 0, 2).reshape(IN_DIM, HN)
    w1n_pad = np.zeros((IN_DIM, NJ * 128), np.float32)
    w1n_pad[:, :HN] = w1n_flat
    w1n = w1n_pad.reshape(KC, 128, NJ, 128).transpose(1, 2, 0, 3)
    w1n = w1n.reshape(128, NJ, KC * 128)
    # block-diagonal node W2 [HN, 126] with columns DUPLICATED (n and 63+n),
    # padded to 1024 rows -> [128, NJ, 126]
    w2bd = np.zeros((NJ * 128, ZW), np.float32)
    for n in range(N_NODES):
        w2bd[n * NODE_HIDDEN : (n + 1) * NODE_HIDDEN, n] = node_w2[n, :, 0]
        w2bd[n * NODE_HIDDEN : (n + 1) * NODE_HIDDEN, N_NODES + n] = node_w2[n, :, 0]
    w2bd = w2bd.reshape(NJ, 128, ZW).transpose(1, 0, 2)
    # node b1 -> [128, NJ]
    b1n = np.zeros((NJ * 128,), np.float32)
    b1n[:HN] = node_b1.reshape(-1)
    b1n = b1n.reshape(NJ, 128).T
    # softplus bias/scale stack: rows 0:63 -> (-b2, -1), rows 63:126 -> (+b2, +1)
    b2 = node_b2[:, 0]
    spsb = np.zeros((128, 2), np.float32)
    spsb[:N_NODES, 0] = -b2
    spsb[:N_NODES, 1] = -1.0
    spsb[N_NODES : 2 * N_NODES, 0] = b2
    spsb[N_NODES : 2 * N_NODES, 1] = 1.0

    # leaf W1 grouped GL leaves per DMA: [NG, 128, GL*KC*128] with
    # (g, p, (i, c, h)) = leaf_w1[g*GL+i, c*128+p, h]
    ng = N_LEAVES // GL
    lw1 = leaf_w1.reshape(ng, GL, KC, 128, LEAF_HIDDEN).transpose(0, 3, 1, 2, 4)
    lw1 = lw1.reshape(ng, 128, GL * KC * 128)
    # leaf W2 grouped: [NG, 128, GL*OUT] with (g, p, (i, o)) = leaf_w2[g*GL+i, p, o]
    lw2 = leaf_w2.reshape(ng, GL, LEAF_HIDDEN, OUT_DIM).transpose(0, 2, 1, 3)
    lw2 = lw2.reshape(ng, 128, GL * OUT_DIM)
    b1l = leaf_b1.T  # [128, 64]

    shared = {
        "w1n": _to_bf16(w1n),
        "w2bd": _to_bf16(w2bd),
        "b1n": np.ascontiguousarray(b1n, np.float32),
        "spsb": spsb,
        "mpath": _path_matrix(),
        "lw1": _to_bf16(lw1),
        "b1l": np.ascontiguousarray(b1l, np.float32),
        "lw2": _to_bf16(lw2),
        "b2l": _to_bf16(leaf_b2),
    }
    xts = []
    for c in range(N_CORES):
        xc = x[c * BC : (c + 1) * BC].T  # [768, 512]
        xt = xc.reshape(KC, 128, BC).transpose(1, 0, 2)
        xts.append(_to_bf16(xt))
    return shared, xts


def kernel(**inputs) -> np.ndarray:
    global LAST_RESULT
    shared, xts = prep_inputs(**inputs)
    nc = _build_nc()
    in_maps = [{**shared, "xt": xts[c]} for c in range(N_CORES)]
    trace = os.environ.get("FFF_TRACE", "0") == "1"
    res = run_bass_kernel_spmd(nc, in_maps, list(range(N_CORES)), trace=trace)
    LAST_RESULT = res
    out = np.empty((BATCH, OUT_DIM), np.float32)
    for c in range(N_CORES):
        out[c * BC : (c + 1) * BC, :] = res.results[c]["outT"].T
    return out
